# revision 4
# baseline (speedup 1.0000x reference)
"""Trainium2 Bass kernel: 3D 'same' convolution (implicit GEMM).

Problem: x (4, 64, 24, 24, 24) f32, weight (1, 128, 1728) f32
         -> out (4, 128, 24, 24, 24) f32  (SAME conv3d, k=3)

Sharding (8 cores): batch (4) x z-halves (2). Each core computes
out[b, :, z0:z0+12] for its (b, zh) shard; no inter-core communication.

Per-core algorithm: 27-tap implicit GEMM in bf16, with taps processed
two-at-a-time per matmul at full K=128 contraction. The PE executes
matmul instructions serially (row-tiled tile_position matmuls do NOT
overlap on HW), so throughput is set purely by (passes x columns).
Packing two taps per pass needs the top 64 partitions to hold the
input window *shifted by the tap delta*: three SBUF tiles carry
replicas shifted by +1x, +1y, +1z. 27 taps = 9 x-pairs (tile A)
+ 3 y-pairs (tile B) + 1 z-pair (tile C) + 1 single (K=64), i.e.
14 serial passes/tile vs 27 before.

Output tiles are one z-plane x 21 y-rows x 24 (N=504); the y=21..23
remainder rows are batched across 6 z-planes (N=432) per half-shard.
Single PSUM accumulation chain per tile; ACT evacuates PSUM->SBUF
with fp32->bf16 downcast; outputs are stored bf16 and upcast on host
(total quantization error ~2e-3 rel, tolerance 2e-2).
"""

import sys

if "/opt/trn_rl_repo" not in sys.path:
    sys.path.insert(0, "/opt/trn_rl_repo")

import ml_dtypes
import numpy as np

CIN, COUT, K = 64, 128, 3
DHW = 24  # cubic spatial extent
ZS = 12  # z-planes per shard
NP = 14  # padded z-planes per shard window (ZS + 2 halo)
PW = 26  # padded y/x extent
N_CORES = 8

# pass table: (tile, tapA, tapB) with tapB = tapA + tile's shift delta;
# tapB None -> single K=64 pass on the tile's bottom half.
# tiles: 0=A (shift +1x), 1=B (+1y), 2=C (+1z)
PASSES = (
    [(0, (dz, dy, 0), (dz, dy, 1)) for dz in range(3) for dy in range(3)]
    + [(1, (dz, 0, 2), (dz, 1, 2)) for dz in range(3)]
    + [(2, (0, 2, 2), (1, 2, 2))]
    + [(0, (2, 2, 2), None)]
)
NPASS = len(PASSES)  # 14


def _build_program(loop_n=None, unroll=False):
    """Build the SPMD Bass program (one NeuronCore's view).

    loop_n: if set, wrap the whole body in a hardware For_i loop with
    that many iterations (used by test.py for wall-clock timing).
    unroll: python-unroll the loop instead (for TimelineSim, which
    can't follow register-mode branches).
    """
    import concourse.tile as tile
    from concourse import bacc, mybir

    BF16 = mybir.dt.bfloat16
    F32 = mybir.dt.float32

    nc = bacc.Bacc("TRN2")
    xa_in = nc.declare_dram_parameter("xa", [128, NP, PW, PW], BF16, isOutput=False)
    xb_in = nc.declare_dram_parameter("xb", [128, NP, PW, PW], BF16, isOutput=False)
    xc_in = nc.declare_dram_parameter("xc", [128, NP, PW, PW], BF16, isOutput=False)
    wk_in = nc.declare_dram_parameter("wk", [128, NPASS, 128], BF16, isOutput=False)
    y_out = nc.declare_dram_parameter("y", [128, ZS, DHW, DHW], BF16, isOutput=True)

    with tile.TileContext(nc) as tc:
        with (
            tc.tile_pool(name="xw", bufs=2) as xw_pool,
            tc.tile_pool(name="ps", bufs=3, space="PSUM") as ps_pool,
            tc.tile_pool(name="ob", bufs=3) as ob_pool,
        ):

            def body(_iv=None):
                W = xw_pool.tile([128, NPASS, 128], BF16, name="W")
                nc.sync.dma_start(out=W[:], in_=wk_in[:])
                XT = []
                for nm, src in (("XA", xa_in), ("XB", xb_in), ("XC", xc_in)):
                    t = xw_pool.tile([128, NP, PW, PW], BF16, name=nm)
                    nc.sync.dma_start(out=t[:], in_=src[:])
                    XT.append(t)

                # output tiles: ("plane", z) N=504 (21x24, 2D AP)
                #            or ("rem", zoff) N=432 (6x3x24, 3D)
                tiles = [("plane", z) for z in range(ZS)] + [("rem", 0), ("rem", 6)]

                def rhs_ap(X, kind, zi, dz, dy, dx, lo, hi):
                    if kind == "plane":
                        return X[lo:hi, zi + dz, dy : dy + 21, dx : dx + 24]
                    return X[lo:hi, zi + dz : zi + dz + 6, 21 + dy : 24 + dy, dx : dx + 24]

                for kind, zi in tiles:
                    n = 504 if kind == "plane" else 432
                    ps = ps_pool.tile([128, 512], F32, name="ps", tag="ps")
                    for j, (ti, ta, tb) in enumerate(PASSES):
                        dz, dy, dx = ta
                        lo, hi = (0, 128) if tb is not None else (0, 64)
                        nc.tensor.matmul(
                            ps[:, :n],
                            lhsT=W[lo:hi, j, :],
                            rhs=rhs_ap(XT[ti], kind, zi, dz, dy, dx, lo, hi),
                            start=(j == 0),
                            stop=(j == NPASS - 1),
                            skip_group_check=True,
                        )
                    ob = ob_pool.tile([128, 512], BF16, name="ob", tag="ob")
                    nc.scalar.copy(ob[:, :n], ps[:, :n])
                    if kind == "plane":
                        nc.sync.dma_start(out=y_out[:, zi, 0:21, :], in_=ob[:, :n])
                    else:
                        # one DMA per z-plane: keeps each transfer one
                        # contiguous run per partition (descriptor-lean)
                        for j in range(6):
                            nc.sync.dma_start(
                                out=y_out[:, zi + j, 21:24, :],
                                in_=ob[:, j * 72 : (j + 1) * 72],
                            )

            if loop_n is not None:
                if unroll:
                    for _k in range(loop_n):
                        body()
                else:
                    with tc.For_i(0, loop_n, 1) as _i:
                        body(_i)
            else:
                body()

    nc.finalize()
    return nc


def _make_in_maps(x, weight):
    w = np.asarray(weight, np.float32).reshape(COUT, CIN, K, K, K)
    wk = np.zeros((128, NPASS, 128), np.float32)
    for j, (_ti, ta, tb) in enumerate(PASSES):
        wk[0:64, j, :] = w[:, :, ta[0], ta[1], ta[2]].T
        if tb is not None:
            wk[64:128, j, :] = w[:, :, tb[0], tb[1], tb[2]].T
    wk = wk.astype(ml_dtypes.bfloat16)

    x = np.asarray(x, np.float32)
    in_maps = []
    for c in range(N_CORES):
        b, zh = divmod(c, 2)
        z0 = zh * ZS
        xpad = np.zeros((CIN, PW, PW, PW), np.float32)
        xpad[:, 1:25, 1:25, 1:25] = x[b]
        win = xpad[:, z0 : z0 + NP]  # (64, 14, 26, 26)

        def repl(shift_axis):
            X = np.zeros((128, NP, PW, PW), np.float32)
            X[0:64] = win
            if shift_axis == 2:  # +1x
                X[64:128, :, :, :-1] = win[:, :, :, 1:]
            elif shift_axis == 1:  # +1y
                X[64:128, :, :-1, :] = win[:, :, 1:, :]
            else:  # +1z
                X[64:128, :-1] = win[:, 1:]
            return X.astype(ml_dtypes.bfloat16)

        in_maps.append(
            {"xa": repl(2), "xb": repl(1), "xc": repl(0), "wk": wk}
        )
    return in_maps


def _gather(results):
    out = np.empty((4, COUT, DHW, DHW, DHW), np.float32)
    for c in range(N_CORES):
        b, zh = divmod(c, 2)
        out[b, :, zh * ZS : (zh + 1) * ZS] = results[c]["y"].astype(np.float32)
    return out


def kernel(x, weight):
    from concourse.bass_utils import run_bass_kernel_spmd

    in_maps = _make_in_maps(x, weight)
    nc = _build_program()
    res = run_bass_kernel_spmd(nc, in_maps, list(range(N_CORES)))
    return _gather(res.results)


# revision 5
# speedup vs baseline: 1.0250x; 1.0250x over previous
"""Trainium2 Bass kernel: 3D 'same' convolution (implicit GEMM).

Problem: x (4, 64, 24, 24, 24) f32, weight (1, 128, 1728) f32
         -> out (4, 128, 24, 24, 24) f32  (SAME conv3d, k=3)

Sharding (8 cores): batch (4) x z-halves (2). Each core computes
out[b, :, z0:z0+12] for its (b, zh) shard; no inter-core communication.

Per-core algorithm: 27-tap implicit GEMM in bf16, with taps processed
two-at-a-time per matmul at full K=128 contraction. PE throughput is
set by the single rhs-streaming XBUS (1 column/cycle total), so the
goal is min(total streamed columns) = ceil(27/2) passes x 6912 output
positions. Packing two taps per pass needs the top 64 partitions to
hold the input window *shifted by the tap delta*: three SBUF tiles
carry replicas shifted by +1x, +1y, +1z. 27 taps = 9 x-pairs (tile A)
+ 3 y-pairs (tile B) + 1 z-pair (tile C) + 1 single (K=64).

Full-row (K=128) matmuls cannot overlap their LDWEIGHTS with the
previous matmul (row groups always conflict), so each weight load is
reused across a group of 7 output tiles (7 PSUM banks in flight,
pass-outer loop) and the redundant legalization-inserted LDWEIGHTS
are elided post-finalize: 196 loads -> ~28.

Output tiles are one z-plane x 21 y-rows x 24 (N=504); the y=21..23
remainder rows are batched across 6 z-planes (N=432) per half-shard.
Group 1 = planes z=0..5 + rem(0..5) reads only padded planes 0..8;
group 2 = planes 6..11 + rem(6..11) reads planes 6..13 — X is loaded
as two overlapping z-chunks so next iteration's chunk-1 DMA overlaps
this iteration's group-2 compute. ACT evacuates PSUM->SBUF with
fp32->bf16 downcast; outputs are stored bf16 and upcast on host
(total quantization error ~3e-3 rel, tolerance 2e-2).
"""

import sys

if "/opt/trn_rl_repo" not in sys.path:
    sys.path.insert(0, "/opt/trn_rl_repo")

import ml_dtypes
import numpy as np

CIN, COUT, K = 64, 128, 3
DHW = 24  # cubic spatial extent
ZS = 12  # z-planes per shard
NP = 14  # padded z-planes per shard window (ZS + 2 halo)
ZA, ZB0, ZB = 9, 6, 8  # chunk1 planes 0..8, chunk2 planes 6..13
PW = 26  # padded y/x extent
N_CORES = 8

# pass table: (tile, tapA, tapB) with tapB = tapA + tile's shift delta;
# tapB None -> single K=64 pass on the tile's bottom half.
# tiles: 0=A (shift +1x), 1=B (+1y), 2=C (+1z)
PASSES = (
    [(0, (dz, dy, 0), (dz, dy, 1)) for dz in range(3) for dy in range(3)]
    + [(1, (dz, 0, 2), (dz, 1, 2)) for dz in range(3)]
    + [(2, (0, 2, 2), (1, 2, 2))]
    + [(0, (2, 2, 2), None)]
)
NPASS = len(PASSES)  # 14


def _elide_redundant_ldweights(nc):
    """Drop legalization-inserted LDWEIGHTS that reload the identical
    weights AP already resident in the PE array (same block, no
    intervening different load). Only sync-free loads are elided."""
    n_drop = 0
    for f in nc.m.functions:
        for b in f.blocks:
            last_key = None
            drop = []
            for inst in b.instructions:
                tn = type(inst).__name__
                if tn == "InstLdweights":
                    key = (str(inst.ins[0]), str(inst.perf_mode), str(inst.is_transpose))
                    si = inst.sync_info
                    clean = si is None or (len(si.on_wait) == 0 and len(si.on_update) == 0)
                    if key == last_key and clean:
                        drop.append(inst)
                    else:
                        last_key = key
            for inst in drop:
                b.instructions.remove(inst)
            n_drop += len(drop)
    return n_drop


def _build_program(loop_n=None, unroll=False):
    """Build the SPMD Bass program (one NeuronCore's view).

    loop_n: if set, wrap the whole body in a hardware For_i loop with
    that many iterations (used by test.py for wall-clock timing).
    unroll: python-unroll the loop instead (for TimelineSim, which
    can't follow register-mode branches).
    """
    import concourse.tile as tile
    from concourse import bacc, mybir

    BF16 = mybir.dt.bfloat16
    F32 = mybir.dt.float32

    nc = bacc.Bacc("TRN2")
    # X inputs: 3 shift-variants x 2 overlapping z-chunks
    xins = []
    for s in "abc":
        x1 = nc.declare_dram_parameter(f"x{s}1", [128, ZA, PW, PW], BF16, isOutput=False)
        x2 = nc.declare_dram_parameter(f"x{s}2", [128, ZB, PW, PW], BF16, isOutput=False)
        xins.append((x1, x2))
    wk_in = nc.declare_dram_parameter("wk", [128, NPASS, 128], BF16, isOutput=False)
    y_out = nc.declare_dram_parameter("y", [128, ZS, DHW, DHW], BF16, isOutput=True)

    with tile.TileContext(nc) as tc:
        with (
            tc.tile_pool(name="xw", bufs=1) as xw_pool,
            tc.tile_pool(name="ps", bufs=8, space="PSUM") as ps_pool,
            tc.tile_pool(name="ob", bufs=4) as ob_pool,
        ):

            def body(_iv=None):
                W = xw_pool.tile([128, NPASS, 128], BF16, name="W", tag="W")
                nc.sync.dma_start(out=W[:], in_=wk_in[:])
                XT1, XT2 = [], []
                for s, (x1, x2) in zip("abc", xins):
                    t1 = xw_pool.tile([128, ZA, PW, PW], BF16, name=f"X{s}1", tag=f"X{s}1")
                    nc.sync.dma_start(out=t1[:], in_=x1[:])
                    XT1.append(t1)
                for s, (x1, x2) in zip("abc", xins):
                    t2 = xw_pool.tile([128, ZB, PW, PW], BF16, name=f"X{s}2", tag=f"X{s}2")
                    nc.sync.dma_start(out=t2[:], in_=x2[:])
                    XT2.append(t2)

                # output tiles: ("plane", z) N=504 (21x24, 2D AP)
                #            or ("rem", zoff) N=432 (6x3x24, 3D)
                # group 1 reads padded planes 0..8 (chunk 1), group 2
                # reads planes 6..13 (chunk 2, stored from plane 6).
                groups = [
                    (XT1, 0, [("plane", z) for z in range(6)] + [("rem", 0)]),
                    (XT2, 6, [("plane", z) for z in range(6, 12)] + [("rem", 6)]),
                ]

                def rhs_ap(X, zbase, kind, zi, dz, dy, dx, lo, hi):
                    if kind == "plane":
                        return X[lo:hi, zi - zbase + dz, dy : dy + 21, dx : dx + 24]
                    z0 = zi - zbase + dz
                    return X[lo:hi, z0 : z0 + 6, 21 + dy : 24 + dy, dx : dx + 24]

                for XT, zbase, gtiles in groups:
                    pss = []
                    for kind, zi in gtiles:
                        ps = ps_pool.tile([128, 512], F32, name="ps", tag="ps")
                        pss.append(ps)
                    for j, (ti, ta, tb) in enumerate(PASSES):
                        dz, dy, dx = ta
                        lo, hi = (0, 128) if tb is not None else (0, 64)
                        for (kind, zi), ps in zip(gtiles, pss):
                            n = 504 if kind == "plane" else 432
                            nc.tensor.matmul(
                                ps[:, :n],
                                lhsT=W[lo:hi, j, :],
                                rhs=rhs_ap(XT[ti], zbase, kind, zi, dz, dy, dx, lo, hi),
                                start=(j == 0),
                                stop=(j == NPASS - 1),
                                skip_group_check=True,
                            )
                    for (kind, zi), ps in zip(gtiles, pss):
                        n = 504 if kind == "plane" else 432
                        ob = ob_pool.tile([128, 512], BF16, name="ob", tag="ob")
                        nc.scalar.copy(ob[:, :n], ps[:, :n])
                        if kind == "plane":
                            nc.sync.dma_start(out=y_out[:, zi, 0:21, :], in_=ob[:, :n])
                        else:
                            # one DMA per z-plane: keeps each transfer one
                            # contiguous run per partition (descriptor-lean)
                            for j in range(6):
                                nc.sync.dma_start(
                                    out=y_out[:, zi + j, 21:24, :],
                                    in_=ob[:, j * 72 : (j + 1) * 72],
                                )

            if loop_n is not None:
                if unroll:
                    for _k in range(loop_n):
                        body()
                else:
                    with tc.For_i(0, loop_n, 1) as _i:
                        body(_i)
            else:
                body()

    nc.finalize()
    _elide_redundant_ldweights(nc)
    return nc


def _make_in_maps(x, weight):
    w = np.asarray(weight, np.float32).reshape(COUT, CIN, K, K, K)
    wk = np.zeros((128, NPASS, 128), np.float32)
    for j, (_ti, ta, tb) in enumerate(PASSES):
        wk[0:64, j, :] = w[:, :, ta[0], ta[1], ta[2]].T
        if tb is not None:
            wk[64:128, j, :] = w[:, :, tb[0], tb[1], tb[2]].T
    wk = wk.astype(ml_dtypes.bfloat16)

    x = np.asarray(x, np.float32)
    in_maps = []
    for c in range(N_CORES):
        b, zh = divmod(c, 2)
        z0 = zh * ZS
        xpad = np.zeros((CIN, PW, PW, PW), np.float32)
        xpad[:, 1:25, 1:25, 1:25] = x[b]
        win = xpad[:, z0 : z0 + NP]  # (64, 14, 26, 26)

        def repl(shift_axis):
            X = np.zeros((128, NP, PW, PW), np.float32)
            X[0:64] = win
            if shift_axis == 2:  # +1x
                X[64:128, :, :, :-1] = win[:, :, :, 1:]
            elif shift_axis == 1:  # +1y
                X[64:128, :, :-1, :] = win[:, :, 1:, :]
            else:  # +1z
                X[64:128, :-1] = win[:, 1:]
            return X.astype(ml_dtypes.bfloat16)

        m = {"wk": wk}
        for s, ax in (("a", 2), ("b", 1), ("c", 0)):
            X = repl(ax)
            m[f"x{s}1"] = np.ascontiguousarray(X[:, 0:ZA])
            m[f"x{s}2"] = np.ascontiguousarray(X[:, ZB0 : ZB0 + ZB])
        in_maps.append(m)
    return in_maps


def _gather(results):
    out = np.empty((4, COUT, DHW, DHW, DHW), np.float32)
    for c in range(N_CORES):
        b, zh = divmod(c, 2)
        out[b, :, zh * ZS : (zh + 1) * ZS] = results[c]["y"].astype(np.float32)
    return out


def kernel(x, weight):
    from concourse.bass_utils import run_bass_kernel_spmd

    in_maps = _make_in_maps(x, weight)
    nc = _build_program()
    res = run_bass_kernel_spmd(nc, in_maps, list(range(N_CORES)))
    return _gather(res.results)
